# revision 2
# baseline (speedup 1.0000x reference)
"""Cox negative log partial likelihood (naive N^2 risk-set formulation) on
8 Trainium2 NeuronCores.

Column sharding (per the problem's sharding hint):
  sum_risk[j] = sum_i exp(log_risk_i) * [ytime_i >= ytime_j]
Each core handles a 2048-wide j-block and all 16384 i's:
  - i laid out as [128 partitions, C chunks], i = p*C + c
  - per i-chunk c: a 0/1 (or sign) mask tile [128, J] is produced on
    VectorE (tensor_scalar is_le) or ScalarE (Sign activation)
  - TensorE reduces over i via matmul accumulate with M=1 risk weights,
    3-way column tiling packs 3 concurrent chunk-matmuls in the PE array
  - epilogue: log(sum_risk), partial = sum((log_risk_j - log_sum_j)*ystatus_j)
    and partial event count; host sums 8 partial pairs -> scalar loss.

V2 fast path: ytime keys are bf16-rounded (host-side, consistently for all
comparisons) so the DVE mask op runs in 4x mode. ScalarE computes a subset of
chunks as sign masks (+1/0/-1) with risk/2 weights, an extra ones-matmul adds
the 0.5*sum(risk_act) constant, and a host-supplied diagonal pattern restores
the exact self-comparison term. Error vs the exact fp32 compare is ~1e-4
relative on the final scalar (spurious bf16 key ties), well under tolerance.
"""

import numpy as np

N = 16384
CORES = 8
P = 128

# --- configuration ---------------------------------------------------------
VERSION = 2          # 1 = exact fp32 compare, DVE-only masks, no col tiling
N_ACT_FRAC = 4       # ACT handles chunks with c % N_ACT_FRAC == N_ACT_FRAC-1
COL_GROUPS = 3       # PE column-tiling groups (1 = off)

_CACHED = {}


def _act_set(C, version):
    if version < 2:
        return set()
    return {c for c in range(C) if c % N_ACT_FRAC == N_ACT_FRAC - 1}


def _build_nc(n=N, cores=CORES, version=VERSION, col_groups=COL_GROUPS):
    from contextlib import ExitStack

    import concourse.tile as tile
    from concourse import bacc, mybir

    f32 = mybir.dt.float32
    bf16 = mybir.dt.bfloat16
    ACT = mybir.ActivationFunctionType
    LN_HALF = -0.6931471805599453  # ln(0.5)

    J = n // cores
    C = n // P
    JT = min(512, J)
    NJT = J // JT
    Q = J // P               # epilogue free dim ([P, Q] j-layout)
    act_set = _act_set(C, version)
    ncols = col_groups if version >= 2 else 1

    nc = bacc.Bacc("TRN2", target_bir_lowering=False, debug=False, num_devices=cores)
    yt_all = nc.dram_tensor("yt_all", [n], f32, kind="ExternalInput")
    lr_all = nc.dram_tensor("lr_all", [n], f32, kind="ExternalInput")
    ytj_in = nc.dram_tensor("ytj_in", [J], bf16 if version >= 2 else f32,
                            kind="ExternalInput")
    lr_j = nc.dram_tensor("lr_j", [J], f32, kind="ExternalInput")
    ys_j = nc.dram_tensor("ys_j", [J], f32, kind="ExternalInput")
    if version >= 2:
        ytbf_all = nc.dram_tensor("ytbf_all", [n], bf16, kind="ExternalInput")
        pat_in = nc.dram_tensor("pat_in", [J], f32, kind="ExternalInput")
    out = nc.dram_tensor("out", [1, 2], f32, kind="ExternalOutput")

    with tile.TileContext(nc) as tc, ExitStack() as ctx:
        singles = ctx.enter_context(tc.tile_pool(name="singles", bufs=1))
        masks = ctx.enter_context(tc.tile_pool(name="masks", bufs=6))
        psum = ctx.enter_context(tc.tile_pool(name="psum", bufs=1, space="PSUM"))
        ep = ctx.enter_context(tc.tile_pool(name="ep", bufs=1))

        # ---- i-side data, layout [P, C]: i = p*C + c -----------------------
        yt_sb = singles.tile([P, C], f32, tag="yt_sb")
        nc.sync.dma_start(out=yt_sb, in_=yt_all.ap().rearrange("(p c) -> p c", c=C))
        lr_sb = singles.tile([P, C], f32, tag="lr_sb")
        nc.sync.dma_start(out=lr_sb, in_=lr_all.ap().rearrange("(p c) -> p c", c=C))
        risk_bf = singles.tile([P, C], bf16, tag="risk_bf")
        nc.scalar.activation(out=risk_bf, in_=lr_sb, func=ACT.Exp)
        if version >= 2:
            ytbf_sb = singles.tile([P, C], bf16, tag="ytbf_sb")
            nc.sync.dma_start(
                out=ytbf_sb, in_=ytbf_all.ap().rearrange("(p c) -> p c", c=C)
            )
            # risk/2 weights for ACT sign chunks
            rhalf_f = singles.tile([P, C], f32, tag="rhalf_f")
            nc.scalar.activation(out=rhalf_f, in_=lr_sb, func=ACT.Exp, bias=LN_HALF)
            rhalf_bf = singles.tile([P, C], bf16, tag="rhalf_bf")
            nc.vector.tensor_copy(rhalf_bf, rhalf_f)
            # w_act[p] = sum_{c in ACT} risk_half[p, c]
            n_act = len(act_set)
            if n_act:
                step = N_ACT_FRAC
                w_act_f = singles.tile([P, 1], f32, tag="w_act_f")
                nc.vector.reduce_sum(
                    w_act_f, rhalf_f[:, step - 1 :: step], axis=mybir.AxisListType.X
                )
                w_act = singles.tile([P, 1], bf16, tag="w_act")
                nc.vector.tensor_copy(w_act, w_act_f)
                ones_jt = singles.tile([P, JT], bf16, tag="ones_jt")
                nc.vector.memset(ones_jt, 1.0)

        # ---- j-side data ---------------------------------------------------
        # broadcast this core's ytime block across all partitions
        ytj_b = singles.tile([P, J], bf16 if version >= 2 else f32, tag="ytj_b")
        nc.sync.dma_start(
            out=ytj_b,
            in_=ytj_in.ap().rearrange("(a j) -> a j", a=1).to_broadcast([P, J]),
        )

        # ---- main loop: masks on DVE/ACT, reduction on PE ------------------
        psums = [
            psum.tile([P, JT], f32, tag=f"ps{jt}", name=f"ps{jt}")
            for jt in range(NJT)
        ]
        first_seen = set()
        last_c = {}
        for c in range(C):
            last_c[c % ncols] = c
        for c in range(C):
            g = c % ncols
            gp = 32 * g
            m = masks.tile([P, J], bf16, tag="m", name="m")
            if c in act_set:
                # sign(yt_i - yt_j) in {-1, 0, 1}; risk/2 weights
                nc.scalar.activation(
                    out=m, in0=None if False else m, in_=ytj_b, func=ACT.Sign,
                    bias=yt_sb[:, c : c + 1], scale=-1.0,
                ) if False else nc.scalar.activation(
                    out=m, in_=ytj_b, func=ACT.Sign,
                    bias=yt_sb[:, c : c + 1], scale=-1.0,
                )
                w = rhalf_bf[:, c : c + 1]
            else:
                scal = ytbf_sb[:, c : c + 1] if version >= 2 else yt_sb[:, c : c + 1]
                nc.vector.tensor_scalar(
                    out=m, in0=ytj_b, scalar1=scal, scalar2=None,
                    op0=mybir.AluOpType.is_le,
                )
                w = risk_bf[:, c : c + 1]
            start = g not in first_seen
            first_seen.add(g)
            is_last = last_c[g] == c
            # chain g=0 also carries the trailing C_act ones-matmuls
            stop = is_last and not (g == 0 and version >= 2 and len(act_set))
            for jt in range(NJT):
                nc.tensor.matmul(
                    psums[jt][gp : gp + 1, :],
                    lhsT=w,
                    rhs=m[:, jt * JT : (jt + 1) * JT],
                    start=start,
                    stop=stop,
                    tile_position=(0, gp) if ncols > 1 else None,
                )
        if version >= 2 and len(act_set):
            for jt in range(NJT):
                nc.tensor.matmul(
                    psums[jt][0:1, :],
                    lhsT=w_act,
                    rhs=ones_jt,
                    start=False,
                    stop=True,
                    tile_position=(0, 0) if ncols > 1 else None,
                )

        # ---- epilogue in [P, Q] j-layout: j = p*Q + q ----------------------
        sumr = ep.tile([P, Q], f32, tag="sumr")
        PB = JT // Q  # partitions covered per psum tile
        for jt in range(NJT):
            for g in range(ncols):
                tgt = sumr[jt * PB : (jt + 1) * PB, :]
                src = psums[jt][32 * g : 32 * g + 1, :]
                if g == 0:
                    nc.sync.dma_start(out=tgt, in_=src)
                else:
                    tmp = ep.tile([P, Q], f32, tag=f"sumr_g{g}", name=f"sumr_g{g}")
                    nc.sync.dma_start(out=tmp[jt * PB : (jt + 1) * PB, :], in_=src)
        for g in range(1, ncols):
            tmp = ep.pool_tiles[f"sumr_g{g}"] if False else None
        # (re-fetch tiles by name is awkward; accumulate instead below)

        lrj_sb = ep.tile([P, Q], f32, tag="lrj")
        nc.sync.dma_start(out=lrj_sb, in_=lr_j.ap().rearrange("(p q) -> p q", q=Q))
        ysj_sb = ep.tile([P, Q], f32, tag="ysj")
        nc.sync.dma_start(out=ysj_sb, in_=ys_j.ap().rearrange("(p q) -> p q", q=Q))
        if version >= 2:
            pat_sb = ep.tile([P, Q], f32, tag="pat")
            nc.sync.dma_start(out=pat_sb, in_=pat_in.ap().rearrange("(p q) -> p q", q=Q))
            riskj = ep.tile([P, Q], f32, tag="riskj")
            nc.scalar.activation(out=riskj, in_=lrj_sb, func=ACT.Exp)
            corr = ep.tile([P, Q], f32, tag="corr")
            nc.vector.tensor_mul(corr, riskj, pat_sb)
            nc.vector.tensor_add(sumr, sumr, corr)

        lsum = ep.tile([P, Q], f32, tag="lsum")
        nc.scalar.activation(out=lsum, in_=sumr, func=ACT.Ln)
        d = ep.tile([P, Q], f32, tag="d")
        nc.vector.tensor_sub(d, lrj_sb, lsum)
        nc.vector.tensor_mul(d, d, ysj_sb)
        red = ep.tile([P, 2], f32, tag="red")
        nc.vector.reduce_sum(red[:, 0:1], d, axis=mybir.AxisListType.X)
        nc.vector.reduce_sum(red[:, 1:2], ysj_sb, axis=mybir.AxisListType.X)
        ones_p = ep.tile([P, 1], f32, tag="ones_p")
        nc.vector.memset(ones_p, 1.0)
        res_ps = psum.tile([1, 2], f32, tag="res_ps", name="res_ps")
        nc.tensor.matmul(res_ps, lhsT=ones_p, rhs=red, start=True, stop=True)
        res = ep.tile([1, 2], f32, tag="res")
        nc.vector.tensor_copy(res, res_ps)
        nc.sync.dma_start(out=out.ap(), in_=res)

    nc.compile()
    return nc


def _get_nc(n=N, cores=CORES, version=VERSION, col_groups=COL_GROUPS):
    key = (n, cores, version, col_groups)
    if key not in _CACHED:
        _CACHED[key] = _build_nc(*key)
    return _CACHED[key]


def _make_in_maps(log_risk, ytime, ystatus, n=N, cores=CORES, version=VERSION):
    import ml_dtypes

    J = n // cores
    C = n // P
    lr = np.ascontiguousarray(np.asarray(log_risk, dtype=np.float32).reshape(-1))
    yt = np.ascontiguousarray(np.asarray(ytime, dtype=np.float32).reshape(-1))
    ys = np.ascontiguousarray(np.asarray(ystatus, dtype=np.float32).reshape(-1))
    assert lr.shape == (n,) and yt.shape == (n,) and ys.shape == (n,)
    if version >= 2:
        ytbf = yt.astype(ml_dtypes.bfloat16)
        act_set = _act_set(C, version)
        pat = np.array(
            [0.5 if (jj % C) in act_set else 0.0 for jj in range(J)],
            dtype=np.float32,
        )
    in_maps = []
    for core in range(cores):
        sl = slice(core * J, (core + 1) * J)
        m = {
            "yt_all": yt,
            "lr_all": lr,
            "lr_j": np.ascontiguousarray(lr[sl]),
            "ys_j": np.ascontiguousarray(ys[sl]),
        }
        if version >= 2:
            m["ytbf_all"] = ytbf
            m["ytj_in"] = np.ascontiguousarray(ytbf[sl])
            m["pat_in"] = pat
        else:
            m["ytj_in"] = np.ascontiguousarray(yt[sl])
        in_maps.append(m)
    return in_maps


def _combine(results, cores=CORES):
    outs = np.stack([results[i]["out"] for i in range(cores)])  # [cores, 1, 2]
    num = outs[:, 0, 0].astype(np.float64).sum()
    den = outs[:, 0, 1].astype(np.float64).sum()
    return np.asarray(-num / den, dtype=np.float32)


def _run(inputs, trace=False):
    """Returns (loss, exec_time_ns or None)."""
    from concourse.bass_utils import run_bass_kernel_spmd

    nc = _get_nc()
    in_maps = _make_in_maps(**inputs)
    res = run_bass_kernel_spmd(nc, in_maps, core_ids=list(range(CORES)), trace=trace)
    return _combine(res.results), res.exec_time_ns


def kernel(log_risk, ytime, ystatus):
    loss, _ = _run(
        {"log_risk": log_risk, "ytime": ytime, "ystatus": ystatus}, trace=False
    )
    return loss


# revision 8
# speedup vs baseline: 1.9670x; 1.9670x over previous
"""Cox negative log partial likelihood (naive N^2 risk-set formulation) on
8 Trainium2 NeuronCores.

Column sharding (per the problem's sharding hint):
  sum_risk[j] = sum_i exp(log_risk_i) * [ytime_i >= ytime_j]
Each core handles a 2048-wide j-block and all 16384 i's:
  - i laid out as [128 partitions, C chunks], i = p*C + c
  - per i-chunk c: a mask tile [128, J] is produced on VectorE
    (tensor_scalar is_le -> 0/1) or ScalarE (Sign activation -> -1/0/+1)
  - TensorE reduces over i via matmul accumulate with M=1 risk-column
    weights; 3-way column tiling runs 3 chunk-matmuls concurrently
  - epilogue: log(sum_risk), partial = sum((log_risk_j - log_sum_j)*ystatus_j)
    plus partial event count; host sums 8 partial pairs -> scalar loss.

V2 fast path details:
  - ytime keys are bf16-rounded host-side and used consistently in every
    comparison, which lets the DVE mask op run in 4x mode. Spurious key
    ties this introduces shift the final scalar by ~1e-4 relative.
  - ScalarE handles chunks c % N_ACT_FRAC == N_ACT_FRAC-1 as sign masks
    with risk/2 weights; a trailing ones-matmul adds the missing
    0.5*sum_{act chunks} risk constant, and a host-built diagonal pattern
    (0.5*risk_j for ACT-owned diagonals) restores the self-comparison term.
"""

import numpy as np

N = 16384
CORES = 8
P = 128

VERSION = 2          # 1 = exact fp32 compare, DVE-only masks, no col tiling
N_ACT_FRAC = 4       # ACT handles chunks with c % N_ACT_FRAC == N_ACT_FRAC-1
COL_GROUPS = 3       # PE column-tiling groups (1 = off)

_CACHED = {}


def _act_set(C, version):
    if version < 2:
        return set()
    return {c for c in range(C) if c % N_ACT_FRAC == N_ACT_FRAC - 1}


def _build_nc(n=N, cores=CORES, version=VERSION, col_groups=COL_GROUPS):
    from contextlib import ExitStack

    import concourse.tile as tile
    from concourse import bacc, mybir

    f32 = mybir.dt.float32
    bf16 = mybir.dt.bfloat16
    ACT = mybir.ActivationFunctionType
    LN_HALF = -0.6931471805599453  # ln(0.5)

    J = n // cores
    C = n // P
    JT = min(512, J)
    NJT = J // JT
    Q = J // P               # epilogue free dim ([P, Q] j-layout)
    PB = JT // Q             # partitions covered per psum tile in epilogue
    act_set = _act_set(C, version)
    ncols = col_groups if version >= 2 else 1
    key_dt = bf16 if version >= 2 else f32

    nc = bacc.Bacc("TRN2", target_bir_lowering=False, debug=False, num_devices=cores)
    yt_all = nc.dram_tensor("yt_all", [n], f32, kind="ExternalInput")
    lr_all = nc.dram_tensor("lr_all", [n], f32, kind="ExternalInput")
    ytj_in = nc.dram_tensor("ytj_in", [J], key_dt, kind="ExternalInput")
    lr_j = nc.dram_tensor("lr_j", [J], f32, kind="ExternalInput")
    ys_j = nc.dram_tensor("ys_j", [J], f32, kind="ExternalInput")
    if version >= 2:
        pat_in = nc.dram_tensor("pat_in", [J], f32, kind="ExternalInput")
    out = nc.dram_tensor("out", [1, 2], f32, kind="ExternalOutput")

    with tile.TileContext(nc) as tc, ExitStack() as ctx:
        singles = ctx.enter_context(tc.tile_pool(name="singles", bufs=1))
        masks = ctx.enter_context(tc.tile_pool(name="masks", bufs=6))
        psum = ctx.enter_context(tc.tile_pool(name="psum", bufs=1, space="PSUM"))
        ep = ctx.enter_context(tc.tile_pool(name="ep", bufs=1))

        # ---- i-side data, layout [P, C]: i = p*C + c -----------------------
        yt_sb = singles.tile([P, C], f32, tag="yt_sb")
        nc.sync.dma_start(out=yt_sb, in_=yt_all.ap().rearrange("(p c) -> p c", c=C))
        lr_sb = singles.tile([P, C], f32, tag="lr_sb")
        nc.sync.dma_start(out=lr_sb, in_=lr_all.ap().rearrange("(p c) -> p c", c=C))
        risk_bf = singles.tile([P, C], bf16, tag="risk_bf")
        nc.scalar.activation(out=risk_bf, in_=lr_sb, func=ACT.Exp)

        n_act = len(act_set)
        if n_act:
            # risk/2 weights for ACT sign chunks
            rhalf_f = singles.tile([P, C], f32, tag="rhalf_f")
            lnhalf = singles.tile([P, 1], f32, tag="lnhalf")
            nc.vector.memset(lnhalf, LN_HALF)
            nc.scalar.activation(out=rhalf_f, in_=lr_sb, func=ACT.Exp, bias=lnhalf)
            rhalf_bf = singles.tile([P, C], bf16, tag="rhalf_bf")
            nc.vector.tensor_copy(rhalf_bf, rhalf_f)
            # w_act[p] = sum_{c in ACT} risk_half[p, c]  (ACT set is c%F==F-1)
            w_act_f = singles.tile([P, 1], f32, tag="w_act_f")
            nc.vector.reduce_sum(
                w_act_f,
                rhalf_f.rearrange("p (a b) -> p a b", b=N_ACT_FRAC)[
                    :, :, N_ACT_FRAC - 1
                ],
                axis=mybir.AxisListType.X,
            )

        # ---- j-side data ---------------------------------------------------
        ytj_b = singles.tile([P, J], key_dt, tag="ytj_b")
        nc.sync.dma_start(
            out=ytj_b,
            in_=ytj_in.ap().rearrange("(a j) -> a j", a=1).to_broadcast([P, J]),
        )

        # ---- main loop: masks on DVE/ACT, reduction on PE ------------------
        psums = [
            psum.tile([P, JT], f32, tag=f"ps{jt}", name=f"ps{jt}")
            for jt in range(NJT)
        ]
        last_c = {}
        for c in range(C):
            last_c[c % ncols] = c
        first_seen = set()
        for c in range(C):
            g = c % ncols
            gp = 32 * g
            m = masks.tile([P, J], bf16, tag="m", name="m")
            if c in act_set:
                # sign(yt_i - yt_j) in {-1, 0, 1}; risk/2 weights
                nc.scalar.activation(
                    out=m, in_=ytj_b, func=ACT.Sign,
                    bias=yt_sb[:, c : c + 1], scale=-1.0,
                )
                w = rhalf_bf[:, c : c + 1]
            else:
                nc.vector.tensor_scalar(
                    out=m, in0=ytj_b, scalar1=yt_sb[:, c : c + 1], scalar2=None,
                    op0=mybir.AluOpType.is_le,
                )
                w = risk_bf[:, c : c + 1]
            start = g not in first_seen
            first_seen.add(g)
            stop = last_c[g] == c
            for jt in range(NJT):
                nc.tensor.matmul(
                    psums[jt][gp : gp + 1, :],
                    lhsT=w,
                    rhs=m[:, jt * JT : (jt + 1) * JT],
                    start=start,
                    stop=stop,
                    tile_position=(0, gp) if ncols > 1 else None,
                )
        # ---- epilogue in [P, Q] j-layout: j = p*Q + q ----------------------
        # stage psum rows (DMA cannot read PSUM), alternating DVE/ACT copies
        st = ep.tile([P, J], f32, tag="st")
        k = 0
        for jt in range(NJT):
            for g in range(ncols):
                src = psums[jt][32 * g : 32 * g + 1, :]
                dst = st[32 * g : 32 * g + 1, jt * JT : (jt + 1) * JT]
                if k % 2 == 0:
                    nc.vector.tensor_copy(dst, src)
                else:
                    nc.scalar.copy(dst, src)
                k += 1
        gtiles = []
        for g in range(ncols):
            gt = ep.tile([P, Q], f32, tag=f"sumr{g}", name=f"sumr{g}")
            gtiles.append(gt)
            nc.sync.dma_start(out=gt, in_=st[32 * g : 32 * g + 1, :])
        sumr = gtiles[0]
        for g in range(1, ncols):
            nc.vector.tensor_add(sumr, sumr, gtiles[g])

        ones_p = ep.tile([P, 1], f32, tag="ones_p")
        nc.vector.memset(ones_p, 1.0)
        if n_act:
            # C_act = 0.5 * sum_{i in ACT chunks} risk_i, added in f32
            caps = psum.tile([1, 1], f32, tag="caps", name="caps")
            nc.tensor.matmul(caps, lhsT=ones_p, rhs=w_act_f, start=True, stop=True)
            cact_sb = ep.tile([1, 1], f32, tag="cact_sb")
            nc.vector.tensor_copy(cact_sb, caps)
            cact_bc = ep.tile([P, 1], f32, tag="cact_bc")
            nc.gpsimd.partition_broadcast(cact_bc, cact_sb)
            nc.vector.tensor_scalar(
                out=sumr, in0=sumr, scalar1=cact_bc, scalar2=None,
                op0=mybir.AluOpType.add,
            )

        lrj_sb = ep.tile([P, Q], f32, tag="lrj")
        nc.sync.dma_start(out=lrj_sb, in_=lr_j.ap().rearrange("(p q) -> p q", q=Q))
        ysj_sb = ep.tile([P, Q], f32, tag="ysj")
        nc.sync.dma_start(out=ysj_sb, in_=ys_j.ap().rearrange("(p q) -> p q", q=Q))
        if version >= 2:
            pat_sb = ep.tile([P, Q], f32, tag="pat")
            nc.sync.dma_start(
                out=pat_sb, in_=pat_in.ap().rearrange("(p q) -> p q", q=Q)
            )
            riskj = ep.tile([P, Q], f32, tag="riskj")
            nc.scalar.activation(out=riskj, in_=lrj_sb, func=ACT.Exp)
            corr = ep.tile([P, Q], f32, tag="corr")
            nc.vector.tensor_mul(corr, riskj, pat_sb)
            nc.vector.tensor_add(sumr, sumr, corr)

        lsum = ep.tile([P, Q], f32, tag="lsum")
        nc.scalar.activation(out=lsum, in_=sumr, func=ACT.Ln)
        d = ep.tile([P, Q], f32, tag="d")
        nc.vector.tensor_sub(d, lrj_sb, lsum)
        nc.vector.tensor_mul(d, d, ysj_sb)
        red = ep.tile([P, 2], f32, tag="red")
        nc.vector.reduce_sum(red[:, 0:1], d, axis=mybir.AxisListType.X)
        nc.vector.reduce_sum(red[:, 1:2], ysj_sb, axis=mybir.AxisListType.X)
        res_ps = psum.tile([1, 2], f32, tag="res_ps", name="res_ps")
        nc.tensor.matmul(res_ps, lhsT=ones_p, rhs=red, start=True, stop=True)
        res = ep.tile([1, 2], f32, tag="res")
        nc.vector.tensor_copy(res, res_ps)
        nc.sync.dma_start(out=out.ap(), in_=res)

    nc.compile()
    return nc


def _get_nc(n=N, cores=CORES, version=VERSION, col_groups=COL_GROUPS):
    key = (n, cores, version, col_groups)
    if key not in _CACHED:
        _CACHED[key] = _build_nc(*key)
    return _CACHED[key]


def _make_in_maps(log_risk, ytime, ystatus, n=N, cores=CORES, version=VERSION):
    import ml_dtypes

    J = n // cores
    C = n // P
    lr = np.ascontiguousarray(np.asarray(log_risk, dtype=np.float32).reshape(-1))
    yt = np.ascontiguousarray(np.asarray(ytime, dtype=np.float32).reshape(-1))
    ys = np.ascontiguousarray(np.asarray(ystatus, dtype=np.float32).reshape(-1))
    assert lr.shape == (n,) and yt.shape == (n,) and ys.shape == (n,)
    if version >= 2:
        ytk = yt.astype(ml_dtypes.bfloat16)
        act_set = _act_set(C, version)
        pat = np.array(
            [0.5 if (jj % C) in act_set else 0.0 for jj in range(J)],
            dtype=np.float32,
        )
    else:
        ytk = yt
    in_maps = []
    for core in range(cores):
        sl = slice(core * J, (core + 1) * J)
        m = {
            "yt_all": ytk.astype(np.float32),
            "lr_all": lr,
            "ytj_in": np.ascontiguousarray(ytk[sl]),
            "lr_j": np.ascontiguousarray(lr[sl]),
            "ys_j": np.ascontiguousarray(ys[sl]),
        }
        if version >= 2:
            m["pat_in"] = pat
        in_maps.append(m)
    return in_maps


def _combine(results, cores=CORES):
    outs = np.stack([results[i]["out"] for i in range(cores)])  # [cores, 1, 2]
    num = outs[:, 0, 0].astype(np.float64).sum()
    den = outs[:, 0, 1].astype(np.float64).sum()
    return np.asarray(-num / den, dtype=np.float32)


def _run(inputs, trace=False):
    """Returns (loss, exec_time_ns or None)."""
    from concourse.bass_utils import run_bass_kernel_spmd

    nc = _get_nc()
    in_maps = _make_in_maps(**inputs)
    res = run_bass_kernel_spmd(nc, in_maps, core_ids=list(range(CORES)), trace=trace)
    return _combine(res.results), res.exec_time_ns


def kernel(log_risk, ytime, ystatus):
    loss, _ = _run(
        {"log_risk": log_risk, "ytime": ytime, "ystatus": ystatus}, trace=False
    )
    return loss


# revision 11
# speedup vs baseline: 2.4507x; 1.2459x over previous
"""Cox negative log partial likelihood (naive N^2 risk-set formulation) on
8 Trainium2 NeuronCores.

Column sharding (per the problem's sharding hint):
  sum_risk[j] = sum_i exp(log_risk_i) * [ytime_i >= ytime_j]
Each core handles a 2048-wide j-block and all 16384 i's:
  - i laid out as [128 partitions, C chunks], i = p*C + c
  - per i-chunk c: a mask tile [128, J] is produced on VectorE
    (tensor_scalar is_le -> 0/1) or ScalarE (Sign activation -> -1/0/+1)
  - TensorE reduces over i via matmul accumulate with M=1 risk-column
    weights; 3-way column tiling runs 3 chunk-matmuls concurrently
  - epilogue: log(sum_risk), partial = sum((log_risk_j - log_sum_j)*ystatus_j)
    plus partial event count; host sums 8 partial pairs -> scalar loss.

V2 fast path details:
  - ytime keys are bf16-rounded host-side and used consistently in every
    comparison, which lets the DVE mask op run in 4x mode. Spurious key
    ties this introduces shift the final scalar by ~1e-4 relative.
  - ScalarE handles chunks c % N_ACT_FRAC == N_ACT_FRAC-1 as sign masks
    with risk/2 weights; a trailing ones-matmul adds the missing
    0.5*sum_{act chunks} risk constant, and a host-built diagonal pattern
    (0.5*risk_j for ACT-owned diagonals) restores the self-comparison term.
"""

import numpy as np

N = 16384
CORES = 8
P = 128

VERSION = 3          # 1 = exact fp32 compare, DVE-only masks, no col tiling
N_ACT_FRAC = 4       # v2: ACT handles chunks with c % N_ACT_FRAC == N_ACT_FRAC-1
N_ACT = 36           # v3: number of ACT-owned chunks (Bresenham-spread)
COL_GROUPS = 3       # PE column-tiling groups (1 = off)
JF = 1280            # v3: compacted (event-only) j-columns per core

_CACHED = {}


def _act_set(C, version):
    if version < 2:
        return set()
    if version >= 3:
        # regular union of strides so w_act reduces with strided APs
        return {c for c in range(C) if c % 4 == 3 or c % 32 == 1}
    return {c for c in range(C) if c % N_ACT_FRAC == N_ACT_FRAC - 1}


def _build_nc(n=N, cores=CORES, version=VERSION, col_groups=COL_GROUPS):
    from contextlib import ExitStack

    import concourse.tile as tile
    from concourse import bacc, mybir

    f32 = mybir.dt.float32
    bf16 = mybir.dt.bfloat16
    ACT = mybir.ActivationFunctionType
    LN_HALF = -0.6931471805599453  # ln(0.5)

    J = (JF * n // N) if version >= 3 else n // cores
    C = n // P
    JT = min(512, J) if version < 3 else J // 4
    NJT = J // JT
    Q = J // P               # epilogue free dim ([P, Q] j-layout)
    PB = JT // Q             # partitions covered per psum tile in epilogue
    act_set = _act_set(C, version)
    ncols = col_groups if version >= 2 else 1
    key_dt = bf16 if version >= 2 else f32

    nc = bacc.Bacc("TRN2", target_bir_lowering=False, debug=False, num_devices=cores)
    yt_all = nc.dram_tensor("yt_all", [n], f32, kind="ExternalInput")
    lr_all = nc.dram_tensor("lr_all", [n], f32, kind="ExternalInput")
    ytj_in = nc.dram_tensor("ytj_in", [J], key_dt, kind="ExternalInput")
    lr_j = nc.dram_tensor("lr_j", [J], f32, kind="ExternalInput")
    ys_j = nc.dram_tensor("ys_j", [J], f32, kind="ExternalInput")
    if version >= 2:
        pat_in = nc.dram_tensor("pat_in", [J], f32, kind="ExternalInput")
    out = nc.dram_tensor("out", [1, 2], f32, kind="ExternalOutput")

    with tile.TileContext(nc) as tc, ExitStack() as ctx:
        singles = ctx.enter_context(tc.tile_pool(name="singles", bufs=1))
        masks = ctx.enter_context(tc.tile_pool(name="masks", bufs=6))
        psum = ctx.enter_context(tc.tile_pool(name="psum", bufs=1, space="PSUM"))
        ep = ctx.enter_context(tc.tile_pool(name="ep", bufs=1))

        # ---- j-key broadcast first: it gates the first mask ops ------------
        ytj_b = singles.tile([P, J], key_dt, tag="ytj_b")
        nc.sync.dma_start(
            out=ytj_b,
            in_=ytj_in.ap().rearrange("(a j) -> a j", a=1).to_broadcast([P, J]),
        )

        # ---- i-side data, layout [P, C]: i = p*C + c -----------------------
        yt_sb = singles.tile([P, C], f32, tag="yt_sb")
        nc.sync.dma_start(out=yt_sb, in_=yt_all.ap().rearrange("(p c) -> p c", c=C))
        lr_sb = singles.tile([P, C], f32, tag="lr_sb")
        nc.sync.dma_start(out=lr_sb, in_=lr_all.ap().rearrange("(p c) -> p c", c=C))
        risk_bf = singles.tile([P, C], bf16, tag="risk_bf")
        nc.scalar.activation(out=risk_bf, in_=lr_sb, func=ACT.Exp)

        n_act = len(act_set)
        if n_act:
            # risk/2 weights for ACT sign chunks
            rhalf_f = singles.tile([P, C], f32, tag="rhalf_f")
            lnhalf = singles.tile([P, 1], f32, tag="lnhalf")
            nc.vector.memset(lnhalf, LN_HALF)
            nc.scalar.activation(out=rhalf_f, in_=lr_sb, func=ACT.Exp, bias=lnhalf)
            rhalf_bf = singles.tile([P, C], bf16, tag="rhalf_bf")
            nc.vector.tensor_copy(rhalf_bf, rhalf_f)
            # w_act[p] = sum_{c in ACT} risk_half[p, c]
            w_act_f = singles.tile([P, 1], f32, tag="w_act_f")
            if version >= 3:
                nc.vector.reduce_sum(
                    w_act_f,
                    rhalf_f.rearrange("p (a b) -> p a b", b=4)[:, :, 3],
                    axis=mybir.AxisListType.X,
                )
                w_act_f2 = singles.tile([P, 1], f32, tag="w_act_f2")
                nc.vector.reduce_sum(
                    w_act_f2,
                    rhalf_f.rearrange("p (a b) -> p a b", b=32)[:, :, 1],
                    axis=mybir.AxisListType.X,
                )
                nc.vector.tensor_add(w_act_f, w_act_f, w_act_f2)
            else:
                nc.vector.reduce_sum(
                    w_act_f,
                    rhalf_f.rearrange("p (a b) -> p a b", b=N_ACT_FRAC)[
                        :, :, N_ACT_FRAC - 1
                    ],
                    axis=mybir.AxisListType.X,
                )

        # ---- main loop: masks on DVE/ACT, reduction on PE ------------------
        psums = [
            psum.tile([P, JT], f32, tag=f"ps{jt}", name=f"ps{jt}")
            for jt in range(NJT)
        ]
        last_c = {}
        for c in range(C):
            last_c[c % ncols] = c
        first_seen = set()
        for c in range(C):
            g = c % ncols
            gp = 32 * g
            m = masks.tile([P, J], bf16, tag="m", name="m")
            if c in act_set:
                # sign(yt_i - yt_j) in {-1, 0, 1}; risk/2 weights
                nc.scalar.activation(
                    out=m, in_=ytj_b, func=ACT.Sign,
                    bias=yt_sb[:, c : c + 1], scale=-1.0,
                )
                w = rhalf_bf[:, c : c + 1]
            else:
                nc.vector.tensor_scalar(
                    out=m, in0=ytj_b, scalar1=yt_sb[:, c : c + 1], scalar2=None,
                    op0=mybir.AluOpType.is_le,
                )
                w = risk_bf[:, c : c + 1]
            start = g not in first_seen
            first_seen.add(g)
            stop = last_c[g] == c
            for jt in range(NJT):
                nc.tensor.matmul(
                    psums[jt][gp : gp + 1, :],
                    lhsT=w,
                    rhs=m[:, jt * JT : (jt + 1) * JT],
                    start=start,
                    stop=stop,
                    tile_position=(0, gp) if ncols > 1 else None,
                )
        # ---- epilogue in [P, Q] j-layout: j = p*Q + q ----------------------
        # stage psum rows (DMA cannot read PSUM), alternating DVE/ACT copies
        st = ep.tile([P, J], f32, tag="st")
        k = 0
        for jt in range(NJT):
            for g in range(ncols):
                src = psums[jt][32 * g : 32 * g + 1, :]
                dst = st[32 * g : 32 * g + 1, jt * JT : (jt + 1) * JT]
                if k % 2 == 0:
                    nc.vector.tensor_copy(dst, src)
                else:
                    nc.scalar.copy(dst, src)
                k += 1
        gtiles = []
        for g in range(ncols):
            gt = ep.tile([P, Q], f32, tag=f"sumr{g}", name=f"sumr{g}")
            gtiles.append(gt)
            nc.sync.dma_start(out=gt, in_=st[32 * g : 32 * g + 1, :])
        sumr = gtiles[0]
        for g in range(1, ncols):
            nc.vector.tensor_add(sumr, sumr, gtiles[g])

        ones_p = ep.tile([P, 1], f32, tag="ones_p")
        nc.vector.memset(ones_p, 1.0)
        if n_act:
            # C_act = 0.5 * sum_{i in ACT chunks} risk_i, added in f32
            caps = psum.tile([1, 1], f32, tag="caps", name="caps")
            nc.tensor.matmul(caps, lhsT=ones_p, rhs=w_act_f, start=True, stop=True)
            cact_sb = ep.tile([1, 1], f32, tag="cact_sb")
            nc.vector.tensor_copy(cact_sb, caps)
            cact_bc = ep.tile([P, 1], f32, tag="cact_bc")
            nc.gpsimd.partition_broadcast(cact_bc, cact_sb)
            nc.vector.tensor_scalar(
                out=sumr, in0=sumr, scalar1=cact_bc, scalar2=None,
                op0=mybir.AluOpType.add,
            )

        lrj_sb = ep.tile([P, Q], f32, tag="lrj")
        nc.sync.dma_start(out=lrj_sb, in_=lr_j.ap().rearrange("(p q) -> p q", q=Q))
        ysj_sb = ep.tile([P, Q], f32, tag="ysj")
        nc.sync.dma_start(out=ysj_sb, in_=ys_j.ap().rearrange("(p q) -> p q", q=Q))
        if version >= 2:
            pat_sb = ep.tile([P, Q], f32, tag="pat")
            nc.sync.dma_start(
                out=pat_sb, in_=pat_in.ap().rearrange("(p q) -> p q", q=Q)
            )
            riskj = ep.tile([P, Q], f32, tag="riskj")
            nc.scalar.activation(out=riskj, in_=lrj_sb, func=ACT.Exp)
            corr = ep.tile([P, Q], f32, tag="corr")
            nc.vector.tensor_mul(corr, riskj, pat_sb)
            nc.vector.tensor_add(sumr, sumr, corr)

        lsum = ep.tile([P, Q], f32, tag="lsum")
        nc.scalar.activation(out=lsum, in_=sumr, func=ACT.Ln)
        d = ep.tile([P, Q], f32, tag="d")
        nc.vector.tensor_sub(d, lrj_sb, lsum)
        nc.vector.tensor_mul(d, d, ysj_sb)
        red = ep.tile([P, 2], f32, tag="red")
        nc.vector.reduce_sum(red[:, 0:1], d, axis=mybir.AxisListType.X)
        nc.vector.reduce_sum(red[:, 1:2], ysj_sb, axis=mybir.AxisListType.X)
        res_ps = psum.tile([1, 2], f32, tag="res_ps", name="res_ps")
        nc.tensor.matmul(res_ps, lhsT=ones_p, rhs=red, start=True, stop=True)
        res = ep.tile([1, 2], f32, tag="res")
        nc.vector.tensor_copy(res, res_ps)
        nc.sync.dma_start(out=out.ap(), in_=res)

    nc.compile()
    return nc


def _get_nc(n=N, cores=CORES, version=VERSION, col_groups=COL_GROUPS):
    key = (n, cores, version, col_groups)
    if key not in _CACHED:
        _CACHED[key] = _build_nc(*key)
    return _CACHED[key]


def _make_in_maps(log_risk, ytime, ystatus, n=N, cores=CORES, version=VERSION):
    """Returns a list of in_map batches (usually one)."""
    import ml_dtypes

    C = n // P
    lr = np.ascontiguousarray(np.asarray(log_risk, dtype=np.float32).reshape(-1))
    yt = np.ascontiguousarray(np.asarray(ytime, dtype=np.float32).reshape(-1))
    ys = np.ascontiguousarray(np.asarray(ystatus, dtype=np.float32).reshape(-1))
    assert lr.shape == (n,) and yt.shape == (n,) and ys.shape == (n,)
    act_set = _act_set(C, version)

    if version >= 2:
        ytk = yt.astype(ml_dtypes.bfloat16)
        yt_full = ytk.astype(np.float32)
    else:
        ytk = yt
        yt_full = yt

    if version >= 3:
        J = JF * n // N
        idx_all = np.nonzero(ys)[0]
        cap = cores * J
        nbatch = max(1, -(-len(idx_all) // cap))
        batches = []
        for b in range(nbatch):
            bi = idx_all[b * cap : (b + 1) * cap]
            pad = cap - len(bi)
            if pad:
                bi = np.concatenate([bi, np.zeros(pad, dtype=bi.dtype)])
            ysb = np.zeros(cap, dtype=np.float32)
            ysb[: cap - pad] = 1.0
            patb = np.where(
                np.isin(bi % C, list(act_set)), 0.5, 0.0
            ).astype(np.float32) if act_set else np.zeros(cap, np.float32)
            in_maps = []
            for core in range(cores):
                sl = slice(core * J, (core + 1) * J)
                in_maps.append(
                    {
                        "yt_all": yt_full,
                        "lr_all": lr,
                        "ytj_in": np.ascontiguousarray(ytk[bi[sl]]),
                        "lr_j": np.ascontiguousarray(lr[bi[sl]]),
                        "ys_j": np.ascontiguousarray(ysb[sl]),
                        "pat_in": np.ascontiguousarray(patb[sl]),
                    }
                )
            batches.append(in_maps)
        return batches

    J = n // cores
    if version >= 2:
        pat = np.array(
            [0.5 if (jj % C) in act_set else 0.0 for jj in range(J)],
            dtype=np.float32,
        )
    in_maps = []
    for core in range(cores):
        sl = slice(core * J, (core + 1) * J)
        m = {
            "yt_all": yt_full,
            "lr_all": lr,
            "ytj_in": np.ascontiguousarray(ytk[sl]),
            "lr_j": np.ascontiguousarray(lr[sl]),
            "ys_j": np.ascontiguousarray(ys[sl]),
        }
        if version >= 2:
            m["pat_in"] = pat
        in_maps.append(m)
    return [in_maps]


def _combine(all_results, cores=CORES):
    num = 0.0
    den = 0.0
    for results in all_results:
        outs = np.stack([results[i]["out"] for i in range(cores)])
        num += outs[:, 0, 0].astype(np.float64).sum()
        den += outs[:, 0, 1].astype(np.float64).sum()
    return np.asarray(-num / den, dtype=np.float32)


def _run(inputs, trace=False):
    """Returns (loss, exec_time_ns or None)."""
    from concourse.bass_utils import run_bass_kernel_spmd

    nc = _get_nc()
    batches = _make_in_maps(**inputs)
    all_results = []
    exec_ns = None
    for in_maps in batches:
        res = run_bass_kernel_spmd(
            nc, in_maps, core_ids=list(range(CORES)), trace=trace
        )
        all_results.append(res.results)
        if res.exec_time_ns is not None:
            exec_ns = res.exec_time_ns if exec_ns is None else exec_ns + res.exec_time_ns
    return _combine(all_results), exec_ns


def kernel(log_risk, ytime, ystatus):
    loss, _ = _run(
        {"log_risk": log_risk, "ytime": ytime, "ystatus": ystatus}, trace=False
    )
    return loss
